# revision 9
# baseline (speedup 1.0000x reference)
"""Trainium2 Bass kernel for 16-head MHA (B=2, T=2048, C=1024).

Sharding: 8 cores = 2 batches x 4 head-groups (4 heads each).
Each core computes, for its batch b and head group g:
  partialT[c, t] = sum_{h in g} wo[:, h].T @ (softmax(qk^T) @ v_h)^T
in fully transposed space (no on-device transposes needed):
  - host passes xT = x[b].T and pre-transposed weight slices
  - qT/kT computed as [d, t]; v as [t, d] (+ ones column per head for the
    softmax denominator); scores computed directly as [tk, tq]
  - exp applied with per-partition (key) mask bias; denominator emerges as
    row 64 of the attn@v_ext matmul output; normalization folded in as a
    K=1 "replicate" matmul + elementwise multiply
  - final projection consumes the [d, t] head outputs as stationary weights
Host adds the 4 partial sums per batch, the wo bias, and the wv_b @ wo.T
constant row (v-bias contribution commutes through softmax normalization).
"""

import sys

sys.path.insert(0, "/opt/trn_rl_repo")

import numpy as np

# ---- problem constants (hardcoded per harness contract) ----
B = 2
T = 2048
C = 1024
NUM_HEADS = 16
G = 4                 # head groups (tensor-parallel dimension)
HPG = NUM_HEADS // G  # 4 heads per core
DH = C // NUM_HEADS   # 64
DC = HPG * DH         # 256 dims per core
VE = HPG * (DH + 1)   # 260: per head 64 v-dims + 1 ones column
N_CORES = B * G       # 8
PAD_ID = 0.0

CH = 512              # tq chunk (one PSUM bank of fp32)
NCH = T // CH         # 4
NT = T // 128         # 16 token tiles
KT = C // 128         # 8 contraction tiles for projections
DM = DC // 128        # 2 m-tiles for q/k


def build_nc(debug=False):
    import concourse.tile as tile
    from concourse import bacc, mybir

    f32 = mybir.dt.float32
    f32r = mybir.dt.float32r
    Exp = mybir.ActivationFunctionType.Exp
    is_equal = mybir.AluOpType.is_equal
    mult = mybir.AluOpType.mult

    nc = bacc.Bacc(
        "TRN2", target_bir_lowering=False, debug=debug, num_devices=N_CORES
    )

    xT_d = nc.dram_tensor("xT", [C, T], f32r, kind="ExternalInput")
    wqT_d = nc.dram_tensor("wqT", [C, DC], f32r, kind="ExternalInput")
    wkT_d = nc.dram_tensor("wkT", [C, DC], f32r, kind="ExternalInput")
    wvT_d = nc.dram_tensor("wvT", [C, VE], f32r, kind="ExternalInput")
    woT_d = nc.dram_tensor("woT", [DC, C], f32r, kind="ExternalInput")
    bq_d = nc.dram_tensor("bq", [DC], f32, kind="ExternalInput")
    ones_d = nc.dram_tensor("ones", [128, DH], f32r, kind="ExternalInput")
    bk_d = nc.dram_tensor("bk", [DC], f32, kind="ExternalInput")
    outT_d = nc.dram_tensor("outT", [C, T], f32, kind="ExternalOutput")

    from contextlib import ExitStack

    with tile.TileContext(nc) as tc, ExitStack() as stack:
        if True:
            persist = stack.enter_context(tc.tile_pool(name="persist", bufs=1))
            psum = stack.enter_context(
                tc.tile_pool(name="psum", bufs=1, space="PSUM")
            )
            # ---------- PSUM slot rotation ----------
            # "sc" tag: 2 x [128, 1024] (2 banks each) for score tiles
            # "un" tags un0..un3: 4 x [128, 512] for attn@v accumulators;
            # phase 1/3 psum tiles rotate through the un tags too.
            _unrot = [0]

            def un_tile(name):
                i = _unrot[0] % 4
                _unrot[0] += 1
                return psum.tile([128, CH], f32, name=name, tag=f"un{i}", bufs=1)

            # ---------- constant / weight loads ----------
            wq_sb, wk_sb, wv_sb = [], [], []
            for k in range(KT):
                wqk = persist.tile([128, DC], f32r, name=f"wq{k}", tag=f"wq{k}")
                nc.sync.dma_start(wqk[:, :], wqT_d.ap()[k * 128:(k + 1) * 128, :])
                wq_sb.append(wqk)
                wkk = persist.tile([128, DC], f32r, name=f"wk{k}", tag=f"wk{k}")
                nc.sync.dma_start(wkk[:, :], wkT_d.ap()[k * 128:(k + 1) * 128, :])
                wk_sb.append(wkk)
                wvk = persist.tile([128, VE], f32r, name=f"wv{k}", tag=f"wv{k}")
                nc.sync.dma_start(wvk[:, :], wvT_d.ap()[k * 128:(k + 1) * 128, :])
                wv_sb.append(wvk)
            wo_sb = []
            for k2 in range(DM):
                wok = persist.tile([128, C], f32r, name=f"wo{k2}", tag=f"wo{k2}")
                nc.sync.dma_start(wok[:, :], woT_d.ap()[k2 * 128:(k2 + 1) * 128, :])
                wo_sb.append(wok)
            bqt, bkt = [], []
            for m in range(DM):
                bqm = persist.tile([128, 1], f32, name=f"bq{m}", tag=f"bq{m}")
                nc.sync.dma_start(
                    bqm[:, :], bq_d.ap()[m * 128:(m + 1) * 128].unsqueeze(1)
                )
                bqt.append(bqm)
                bkm = persist.tile([128, 1], f32, name=f"bk{m}", tag=f"bk{m}")
                nc.sync.dma_start(
                    bkm[:, :], bk_d.ap()[m * 128:(m + 1) * 128].unsqueeze(1)
                )
                bkt.append(bkm)

            # key-pad mask bias: mb[p, t] = -1e30 if x[t*128+p, 0] == 0 else 0
            xc0 = persist.tile([128, NT], f32r, name="xc0", tag="xc0")
            nc.sync.dma_start(
                xc0[:, :],
                xT_d.ap()[0:1, :].rearrange("a (t p) -> (a p) t", p=128),
            )
            mb = persist.tile([128, NT], f32, name="mb", tag="mb")
            nc.vector.tensor_scalar(
                out=mb[:, :], in0=xc0[:, :], scalar1=0.0, scalar2=-1e30,
                op0=is_equal, op1=mult,
            )

            ones64 = persist.tile([1, DH], f32r, name="ones64", tag="ones64")
            nc.sync.dma_start(ones64[:, :], ones_d.ap()[0:1, :])

            # ---------- phase 1: projections (x tiles freed afterwards) ----------
            qT = [
                persist.tile([128, T], f32r, name=f"qT{m}", tag=f"qT{m}")
                for m in range(DM)
            ]
            kT = [
                persist.tile([128, T], f32r, name=f"kT{m}", tag=f"kT{m}")
                for m in range(DM)
            ]
            v_sb = []
            with tc.tile_pool(name="xpool", bufs=1) as xpool:
                xs = []
                for k in range(KT):
                    xk = xpool.tile([128, T], f32r, name=f"x{k}", tag=f"x{k}")
                    nc.sync.dma_start(xk[:, :], xT_d.ap()[k * 128:(k + 1) * 128, :])
                    xs.append(xk)

                for dst, w_sb, bias in ((qT, wq_sb, bqt), (kT, wk_sb, bkt)):
                    for m in range(DM):
                        for ch in range(NCH):
                            ps = un_tile(f"ps{dst[0].name}{m}{ch}")
                            for k in range(KT):
                                nc.tensor.matmul(
                                    ps[:, :],
                                    w_sb[k][:, m * 128:(m + 1) * 128],
                                    xs[k][:, ch * CH:(ch + 1) * CH],
                                    start=(k == 0),
                                    stop=(k == KT - 1),
                                )
                            nc.vector.tensor_scalar_add(
                                dst[m][:, ch * CH:(ch + 1) * CH],
                                ps[:, :],
                                bias[m][:, :],
                            )

                for tkt in range(NT):
                    psv = un_tile(f"psv{tkt}")
                    for k in range(KT):
                        nc.tensor.matmul(
                            psv[:, 0:VE],
                            xs[k][:, tkt * 128:(tkt + 1) * 128],
                            wv_sb[k][:, :],
                            start=(k == 0),
                            stop=(k == KT - 1),
                        )
                    vt = persist.tile([128, VE], f32r, name=f"v{tkt}", tag=f"v{tkt}")
                    nc.vector.tensor_copy(vt[:, :], psv[:, 0:VE])
                    # ones columns at 64, 129, 194, 259 (one per head)
                    ones_cols = vt.rearrange("p (h e) -> p h e", e=DH + 1)[:, :, DH]
                    nc.sync.dma_start(ones_cols, ones_d.ap()[:, 0:HPG])
                    v_sb.append(vt)

            # ---------- phase 2: attention (per head) ----------
            headsT = [
                persist.tile([128, T], f32r, name=f"headsT{m}", tag=f"hT{m}")
                for m in range(DM)
            ]
            atpool = stack.enter_context(tc.tile_pool(name="atpool", bufs=1))
            work = stack.enter_context(tc.tile_pool(name="work", bufs=1))

            for h in range(HPG):
                m, base = h // 2, (h % 2) * 64
                un = [un_tile(f"unh{h}c{ch}") for ch in range(NCH)]

                def emit_scores(tkt, half):
                    sc = psum.tile(
                        [128, 2 * CH], f32, name=f"sc{h}t{tkt}h{half}",
                        tag=f"sc{half}", bufs=1,
                    )
                    for sub in range(2):
                        ch = half * 2 + sub
                        nc.tensor.matmul(
                            sc[:, sub * CH:(sub + 1) * CH],
                            kT[m][base:base + 64, tkt * 128:(tkt + 1) * 128]
                            ,
                            qT[m][base:base + 64, ch * CH:(ch + 1) * CH]
                            ,
                            start=True,
                            stop=True,
                        )
                    return sc

                # software-pipelined emission: while ACT runs exp of round t,
                # PE runs the attn@v of round t and the scores of round t+1.
                sc_half = [emit_scores(0, 0), emit_scores(0, 1)]
                for tkt in range(NT):
                    for half in range(2):
                        at = atpool.tile(
                            [128, 2 * CH], f32r, name=f"at{h}t{tkt}h{half}",
                            tag=f"at{half}", bufs=3,
                        )
                        nc.scalar.activation(
                            at[:, :], sc_half[half][:, :], Exp,
                            bias=mb[:, tkt:tkt + 1],
                        )
                        for sub in range(2):
                            ch = half * 2 + sub
                            nc.tensor.matmul(
                                un[ch][0:DH + 1, :],
                                v_sb[tkt][:, h * (DH + 1):(h + 1) * (DH + 1)]
                                ,
                                at[:, sub * CH:(sub + 1) * CH],
                                start=(tkt == 0),
                                stop=(tkt == NT - 1),
                            )
                        if tkt + 1 < NT:
                            sc_half[half] = emit_scores(tkt + 1, half)
                # normalize: heads[d, tq] = un[d, tq] / un[64, tq]
                for ch in range(NCH):
                    unev = work.tile(
                        [DH + 1, CH], f32, name=f"unev{h}{ch}", tag="unev", bufs=2
                    )
                    nc.vector.tensor_copy(unev[:, :], un[ch][0:DH + 1, :])
                    dr = work.tile([1, CH], f32, name=f"dr{h}{ch}", tag="dr", bufs=2)
                    nc.sync.dma_start(dr[:, :], unev[DH:DH + 1, :])
                    rr = work.tile([1, CH], f32r, name=f"rr{h}{ch}", tag="rr", bufs=2)
                    with nc.allow_low_precision(reason="fp32r matmul operand"):
                        nc.vector.reciprocal(rr[:, :], dr[:, :])
                    rb = un_tile(f"rb{h}{ch}")
                    nc.tensor.matmul(
                        rb[0:DH, :],
                        ones64[:, :],
                        rr[:, :],
                        start=True,
                        stop=True,
                    )
                    if base == 0:
                        nc.vector.tensor_mul(
                            headsT[m][0:DH, ch * CH:(ch + 1) * CH],
                            unev[0:DH, :],
                            rb[0:DH, :],
                        )
                    else:
                        scr = work.tile(
                            [DH, CH], f32r, name=f"scr{h}{ch}", tag="scr", bufs=2
                        )
                        nc.vector.tensor_mul(scr[:, :], unev[0:DH, :], rb[0:DH, :])
                        nc.sync.dma_start(
                            headsT[m][base:base + 64, ch * CH:(ch + 1) * CH],
                            scr[:, :],
                        )

            # ---------- phase 3: output projection ----------
            for mc in range(C // 128):
                for ch in range(NCH):
                    pp = un_tile(f"pp{mc}{ch}")
                    for k2 in range(DM):
                        nc.tensor.matmul(
                            pp[:, :],
                            wo_sb[k2][:, mc * 128:(mc + 1) * 128],
                            headsT[k2][:, ch * CH:(ch + 1) * CH],
                            start=(k2 == 0),
                            stop=(k2 == DM - 1),
                        )
                    po = work.tile([128, CH], f32, name=f"po{mc}{ch}", tag="po", bufs=3)
                    nc.scalar.copy(po[:, :], pp[:, :])
                    nc.sync.dma_start(
                        outT_d.ap()[mc * 128:(mc + 1) * 128, ch * CH:(ch + 1) * CH],
                        po[:, :],
                    )

    nc.compile()
    return nc


def make_in_maps(x, wq_w, wq_b, wk_w, wk_b, wv_w, wv_b, wo_w, wo_b):
    scale = DH ** -0.5
    in_maps = []
    for c in range(N_CORES):
        b, g = divmod(c, G)
        sl = slice(g * DC, (g + 1) * DC)
        wvT_ext = np.zeros((C, VE), np.float32)
        for hl in range(HPG):
            rows = slice(g * DC + hl * DH, g * DC + (hl + 1) * DH)
            wvT_ext[:, hl * (DH + 1):hl * (DH + 1) + DH] = wv_w[rows, :].T
        in_maps.append({
            "xT": np.ascontiguousarray(x[b].T),
            "wqT": np.ascontiguousarray((wq_w[sl] * scale).T),
            "wkT": np.ascontiguousarray(wk_w[sl].T),
            "wvT": wvT_ext,
            "woT": np.ascontiguousarray(wo_w[:, sl].T),
            "bq": np.ascontiguousarray(wq_b[sl] * scale).astype(np.float32),
            "ones": np.ones((128, DH), np.float32),
            "bk": np.ascontiguousarray(wk_b[sl]).astype(np.float32),
        })
    return in_maps


def assemble_output(results, wv_b, wo_w, wo_b):
    const_row = wv_b @ wo_w.T + wo_b  # [C]
    out = np.zeros((B, T, C), np.float32)
    for c in range(N_CORES):
        b = c // G
        out[b] += results[c]["outT"].T
    out += const_row[None, None, :]
    return out.astype(np.float32)


_nc_cache = {}


def kernel(**inputs):
    from concourse.bass_utils import run_bass_kernel_spmd

    if "nc" not in _nc_cache:
        _nc_cache["nc"] = build_nc(debug=False)
    nc = _nc_cache["nc"]

    in_maps = make_in_maps(**inputs)
    res = run_bass_kernel_spmd(nc, in_maps, core_ids=list(range(N_CORES)))
    return assemble_output(
        res.results, inputs["wv_b"], inputs["wo_w"], inputs["wo_b"]
    )
